# revision 1
# baseline (speedup 1.0000x reference)
"""Trainium2 Bass kernel for nn_ConstraintsModule.

Reference math:
    m = preds[:, atoms]                                   # [B, N]
    body_rev[b,c,j] = pos_body[c,j] + m[b,j]*(neg_body-pos_body)[c,j]
    body_min[b,c]   = 1 - max_j body_rev[b,c,j]
    lb[b,n] = max_c body_min[b,c]*pos_head[c,n]
    ub[b,n] = 1 - max_c body_min[b,c]*neg_head[c,n]
    updated = clamp(m, min(lb,ub), max(lb,ub))
    out = preds with columns `atoms` replaced by updated

Structure exploited:
  * body masks are ~2% dense: max_j body_rev collapses to
    max(1 - min_{j in pos} m, max_{j in neg} m) over ~20 literals.
  * head rows are one-hot: lb/ub are segment maxes of body_min grouped by
    (head atom, sign).

Layout: all 128 batch rows on the SBUF partition axis.  The host packs,
per constraint slot, pos-literal m values (padded with 1.0) and
neg-literal m values (padded with 0.0); slots are grouped into a "light"
region (small uniform width) and a "heavy" region (full width) to cut
padding bytes.  Device work is pure DVE: strided tensor_reduce min/max
per region chunk (overlapped with the chunk DMAs), an exact
body_min = 1-max(1-minP, maxQ) rewrite, per-(atom-group) segment maxes,
and the final clamp.  Every op rounds exactly like the reference, so the
result is bit-identical to the fp32 reference.

Sharding: atoms are grouped by (heavy, pos-bin-size, neg-bin-size) and
dealt round-robin to the 8 cores, so all cores share one SPMD program
(groups padded to the cross-core max count); only packed data differs.
"""

import sys
from contextlib import ExitStack

import numpy as np

if "/opt/trn_rl_repo" not in sys.path:
    sys.path.insert(0, "/opt/trn_rl_repo")

import concourse.bacc as bacc
import concourse.tile as tile
from concourse import mybir
from concourse.bass_utils import run_bass_kernel_spmd

B = 128
C = 1024
N = 512
NCORES = 8
N_LIGHT_CHUNKS = 5

# Set by test.py to profile; the grading path leaves these alone.
_TRACE = False
_LAST_RESULTS = None

_PROGRAM_CACHE: dict = {}


def _roundup(x: int, mult: int) -> int:
    return ((x + mult - 1) // mult) * mult


def _chunk_plan(kpl, knl, kph, knh, sl_pad, sh_pad):
    """Graded chunks (small first, so the first DVE reduce can start as
    early as possible) over [light slots][heavy slots], alternating between
    the two fast HWDGE queues (scalar, gpsimd)."""
    wl, wh = kpl + knl, kph + knh
    work = [("l", sl_pad, wl)]
    if sh_pad:
        work.append(("h", sh_pad, wh))
    total = sl_pad * wl + sh_pad * wh
    # Graded: small first (early DVE start), small last (short post-DMA tail).
    fracs = [0.07, 0.09, 0.13, 0.18, 0.24, 0.21, 0.08]
    bounds = []
    acc = 0.0
    for f in fracs[:-1]:
        acc += f
        bounds.append(int(total * acc))
    chunks = []  # (region, s0, s1)
    done = 0
    for reg, nslots, w in work:
        s = 0
        while s < nslots:
            nxt = [b for b in bounds if b > done]
            budget = (nxt[0] - done) if nxt else (total - done)
            take = min(max(budget // w, 1), nslots - s)
            chunks.append((reg, s, s + take))
            s += take
            done += take * w
    return tuple(chunks)


def _build_program(dims, groups):
    """dims = (kpl, knl, kph, knh, sl_pad, sh_pad, nl_pad);
    groups: tuple of (sp, sn, cnt, col_off, slot_off) in the combined slot
    space (light slots first, then heavy)."""
    key = (dims, groups)
    if key in _PROGRAM_CACHE:
        return _PROGRAM_CACHE[key]
    kpl, knl, kph, knh, sl_pad, sh_pad, nl_pad = dims

    dt = mybir.dt
    wl, wh = kpl + knl, kph + knh
    s_tot = sl_pad + sh_pad
    chunks = _chunk_plan(kpl, knl, kph, knh, sl_pad, sh_pad)

    nc = bacc.Bacc(
        "TRN2", target_bir_lowering=False, debug=False, enable_partition_id=False
    )
    c_ds = [
        nc.dram_tensor(
            f"c{i}", [B, (s1 - s0) * (wl if reg == "l" else wh)], dt.float32,
            kind="ExternalInput",
        )
        for i, (reg, s0, s1) in enumerate(chunks)
    ]
    mloc_d = nc.dram_tensor("mloc", [B, nl_pad], dt.float32, kind="ExternalInput")
    out_d = nc.dram_tensor("upd", [B, nl_pad], dt.float32, kind="ExternalOutput")

    with ExitStack() as ctx:
        tc = ctx.enter_context(tile.TileContext(nc))
        pool = ctx.enter_context(tc.tile_pool(name="main", bufs=1))

        mloc_sb = pool.tile([B, nl_pad], dt.float32, tag="mloc")
        nc.sync.dma_start(mloc_sb[:], mloc_d.ap())

        gl_sb = pool.tile([B, sl_pad * wl], dt.float32, tag="gl")
        gh_sb = pool.tile([B, max(sh_pad, 1) * wh], dt.float32, tag="gh")
        minp_sb = pool.tile([B, s_tot], dt.float32, tag="minp")
        maxq_sb = pool.tile([B, s_tot], dt.float32, tag="maxq")
        # G rides only the two fast HWDGE queues; sync carries mloc/out.
        dma_engines = [nc.scalar, nc.gpsimd]
        for i, (reg, s0, s1) in enumerate(chunks):
            w, kp_w, g_t, base = (
                (wl, kpl, gl_sb, 0) if reg == "l" else (wh, kph, gh_sb, sl_pad)
            )
            dma_engines[i % 2].dma_start(g_t[:, s0 * w : s1 * w], c_ds[i].ap())
            g3 = g_t[:, s0 * w : s1 * w].rearrange("p (c k) -> p c k", k=w)
            nc.vector.tensor_reduce(
                minp_sb[:, base + s0 : base + s1], g3[:, :, 0:kp_w],
                axis=mybir.AxisListType.X, op=mybir.AluOpType.min,
            )
            nc.vector.tensor_reduce(
                maxq_sb[:, base + s0 : base + s1], g3[:, :, kp_w:w],
                axis=mybir.AxisListType.X, op=mybir.AluOpType.max,
            )

        # body_min = 1 - max(1 - minP, maxQ), rounded exactly like the
        # reference (which materializes each 1-m and 1-body_max).
        bmin_sb = pool.tile([B, s_tot], dt.float32, tag="bmin")
        nc.vector.tensor_scalar(
            minp_sb[:], minp_sb[:], -1.0, 1.0,
            op0=mybir.AluOpType.mult, op1=mybir.AluOpType.add,
        )
        nc.vector.tensor_tensor(
            minp_sb[:], minp_sb[:], maxq_sb[:], op=mybir.AluOpType.max
        )
        nc.vector.tensor_scalar(
            bmin_sb[:], minp_sb[:], -1.0, 1.0,
            op0=mybir.AluOpType.mult, op1=mybir.AluOpType.add,
        )

        # Head phase: segment maxes over (atom, sign) bins.
        lb_sb = pool.tile([B, nl_pad], dt.float32, tag="lb")
        ubm_sb = pool.tile([B, nl_pad], dt.float32, tag="ubm")
        nc.vector.memset(lb_sb[:], 0.0)
        nc.vector.memset(ubm_sb[:], 0.0)
        for sp, sn, cnt, col_off, slot_off in groups:
            w = sp + sn
            if w == 0:
                continue  # lb/ubm stay 0 from the memset
            seg = bmin_sb[:, slot_off : slot_off + cnt * w].rearrange(
                "p (a l) -> p a l", l=w
            )
            if sp > 0:
                nc.vector.tensor_reduce(
                    lb_sb[:, col_off : col_off + cnt], seg[:, :, 0:sp],
                    axis=mybir.AxisListType.X, op=mybir.AluOpType.max,
                )
            if sn > 0:
                nc.vector.tensor_reduce(
                    ubm_sb[:, col_off : col_off + cnt], seg[:, :, sp:w],
                    axis=mybir.AxisListType.X, op=mybir.AluOpType.max,
                )

        # updated = max(min(lb, ub), min(max(lb, ub), m)),  ub = 1 - ubm
        ub_sb = pool.tile([B, nl_pad], dt.float32, tag="ub")
        nc.vector.tensor_scalar(
            ub_sb[:], ubm_sb[:], -1.0, 1.0,
            op0=mybir.AluOpType.mult, op1=mybir.AluOpType.add,
        )
        lo_sb = pool.tile([B, nl_pad], dt.float32, tag="lo")
        nc.vector.tensor_tensor(lo_sb[:], lb_sb[:], ub_sb[:], op=mybir.AluOpType.min)
        hi_sb = pool.tile([B, nl_pad], dt.float32, tag="hi")
        nc.vector.tensor_tensor(hi_sb[:], lb_sb[:], ub_sb[:], op=mybir.AluOpType.max)
        upd_sb = pool.tile([B, nl_pad], dt.float32, tag="upd")
        nc.vector.tensor_tensor(upd_sb[:], hi_sb[:], mloc_sb[:], op=mybir.AluOpType.min)
        nc.vector.tensor_tensor(upd_sb[:], lo_sb[:], upd_sb[:], op=mybir.AluOpType.max)
        nc.sync.dma_start(out_d.ap(), upd_sb[:])

    nc.compile()
    _PROGRAM_CACHE[key] = nc
    return nc


def kernel(preds, pos_head, neg_head, pos_body, neg_body, atoms):
    global _LAST_RESULTS
    preds = np.ascontiguousarray(np.asarray(preds, dtype=np.float32))
    pos_head = np.asarray(pos_head)
    neg_head = np.asarray(neg_head)
    pos_body = np.asarray(pos_body)
    neg_body = np.asarray(neg_body)
    atoms_np = np.asarray(atoms).astype(np.int64)

    m = np.ascontiguousarray(preds[:, atoms_np].astype(np.float32))  # [B, N]
    # m_ext columns: [0..N) = m, N = 1.0 (pos pad), N+1 = 0.0 (neg/dummy pad)
    m_ext = np.concatenate(
        [m, np.ones((B, 1), np.float32), np.zeros((B, 1), np.float32)], axis=1
    )
    POS_PAD, NEG_PAD = N, N + 1

    pb = pos_body != 0
    nb_ = neg_body != 0
    kp_c = pb.sum(1)
    kn_c = nb_.sum(1)
    kph = max(_roundup(int(kp_c.max()), 4), 4)
    knh = max(_roundup(int(kn_c.max()), 4), 4)

    body_js = [
        (np.nonzero(pb[c])[0], np.nonzero(nb_[c])[0]) for c in range(C)
    ]

    # Head occurrences: one slot per (constraint, sign) head.
    ph_atom = pos_head.argmax(1)
    ph_has = pos_head.max(1) > 0
    nh_atom = neg_head.argmax(1)
    nh_has = neg_head.max(1) > 0
    pos_bins = [[] for _ in range(N)]
    neg_bins = [[] for _ in range(N)]
    for c in np.nonzero(ph_has)[0]:
        pos_bins[ph_atom[c]].append(c)
    for c in np.nonzero(nh_has)[0]:
        neg_bins[nh_atom[c]].append(c)

    # Per-atom max body widths over its bins' constraints.
    atom_kp = np.zeros(N, np.int64)
    atom_kn = np.zeros(N, np.int64)
    for n in range(N):
        cs = pos_bins[n] + neg_bins[n]
        if cs:
            atom_kp[n] = max(kp_c[c] for c in cs)
            atom_kn[n] = max(kn_c[c] for c in cs)

    # Pick light-tier thresholds + bin-size bucketing minimizing the true
    # per-core packed bytes (cross-core ceil padding included).  Bucketing
    # bins up to a multiple of bb adds dummy all-zero constraint slots
    # (bmin=0, neutral in the bin max) but merges groups, cutting both the
    # ceil padding and the head-phase instruction count.
    from collections import Counter, defaultdict

    nsp = np.array([len(pos_bins[n]) for n in range(N)])
    nsn = np.array([len(neg_bins[n]) for n in range(N)])

    def structure(kpl_, knl_, bb):
        heavy = (atom_kp > kpl_) | (atom_kn > knl_)
        cnt = Counter()
        keys = []
        for n in range(N):
            spb = -(-int(nsp[n]) // bb) * bb if nsp[n] else 0
            snb = -(-int(nsn[n]) // bb) * bb if nsn[n] else 0
            key = (bool(heavy[n]), spb, snb)
            keys.append(key)
            cnt[key] += 1
        cost = sum(
            -(-c // NCORES) * (kk[1] + kk[2]) * ((kph + knh) if kk[0] else (kpl_ + knl_))
            for kk, c in cnt.items()
        )
        return cost, cnt, keys

    best = None
    for kpl_c in (8, 12, 16, 20, kph):
        for knl_c in (8, 12, 16, 20, 24, knh):
            for bb in (1, 2, 4):
                cost, cnt, keys = structure(kpl_c, knl_c, bb)
                rank = (cost, len(cnt) * 8)
                if best is None or rank < best[0]:
                    best = (rank, kpl_c, knl_c, bb, keys)
    _, kpl, knl, bb, atom_keys = best
    wl, wh = kpl + knl, kph + knh

    group_atoms = defaultdict(list)
    for n in range(N):
        group_atoms[atom_keys[n]].append(n)

    # Light groups first: slot index space is [light slots][heavy slots].
    gkeys = sorted(group_atoms)  # False < True
    n_light_slots = sum(
        -(-len(group_atoms[k]) // NCORES) * (k[1] + k[2]) for k in gkeys if not k[0]
    )
    sl_pad = _roundup(max(n_light_slots, N_LIGHT_CHUNKS), N_LIGHT_CHUNKS)

    groups = []  # (sp, sn, cnt, col_off, slot_off) in combined slot space
    core_atoms = [[] for _ in range(NCORES)]  # (group_idx, pos_in_group, atom)
    col_off = 0
    slot_l = 0
    slot_h = sl_pad
    for key in gkeys:
        heavy, sp, sn = key
        atoms_g = group_atoms[key]
        cnt = -(-len(atoms_g) // NCORES)
        for j, a in enumerate(atoms_g):
            core_atoms[j % NCORES].append((len(groups), j // NCORES, a))
        soff = slot_h if heavy else slot_l
        groups.append((sp, sn, cnt, col_off, soff))
        col_off += cnt
        if heavy:
            slot_h += cnt * (sp + sn)
        else:
            slot_l += cnt * (sp + sn)
    assert slot_l <= sl_pad
    sh_pad = _roundup(slot_h - sl_pad, 2)
    nl_pad = _roundup(col_off, 4)

    dims = (kpl, knl, kph, knh, sl_pad, sh_pad, nl_pad)
    nc = _build_program(dims, tuple(groups))

    in_maps = []
    out_cols = []  # per core: (cols, atom_ids) to scatter back
    for core in range(NCORES):
        light_rows = np.full((sl_pad, wl), NEG_PAD, np.int32)
        heavy_rows = np.full((max(sh_pad, 1), wh), NEG_PAD, np.int32)
        mloc_idx = np.full(nl_pad, NEG_PAD, np.int32)  # dummy -> 0.0
        cols = []
        atom_ids = []
        for gi, pos_in_g, a in core_atoms[core]:
            sp, sn, cnt, coff, soff = groups[gi]
            heavy = soff >= sl_pad
            rows, kp_w, base0 = (
                (heavy_rows, kph, soff - sl_pad) if heavy else (light_rows, kpl, soff)
            )
            base = base0 + pos_in_g * (sp + sn)
            for l, cid in enumerate(pos_bins[a]):
                jp, jn = body_js[cid]
                rows[base + l, : jp.size] = jp
                rows[base + l, jp.size : kp_w] = POS_PAD
                rows[base + l, kp_w : kp_w + jn.size] = jn
            for l, cid in enumerate(neg_bins[a]):
                jp, jn = body_js[cid]
                rows[base + sp + l, : jp.size] = jp
                rows[base + sp + l, jp.size : kp_w] = POS_PAD
                rows[base + sp + l, kp_w : kp_w + jn.size] = jn
            mloc_idx[coff + pos_in_g] = a
            cols.append(coff + pos_in_g)
            atom_ids.append(a)
        gl_vals = m_ext[:, light_rows.ravel()]
        gh_vals = m_ext[:, heavy_rows.ravel()]
        chunks = _chunk_plan(kpl, knl, kph, knh, sl_pad, sh_pad)
        im = {}
        for i, (reg, s0, s1) in enumerate(chunks):
            vals, w = (gl_vals, wl) if reg == "l" else (gh_vals, wh)
            im[f"c{i}"] = np.ascontiguousarray(vals[:, s0 * w : s1 * w])
        im["mloc"] = np.ascontiguousarray(m_ext[:, mloc_idx])
        in_maps.append(im)
        out_cols.append((np.array(cols), np.array(atom_ids)))

    res = run_bass_kernel_spmd(
        nc, in_maps, core_ids=list(range(NCORES)), trace=_TRACE
    )
    _LAST_RESULTS = res

    out = preds.copy()
    for core in range(NCORES):
        cols, atom_ids = out_cols[core]
        if len(cols):
            out[:, atoms_np[atom_ids]] = res.results[core]["upd"][:, cols]
    return out



# revision 3
# speedup vs baseline: 1.3312x; 1.3312x over previous
"""Trainium2 Bass kernel for nn_ConstraintsModule (v2: min-form + PE scatter).

Reference math:
    m = preds[:, atoms]                                   # [B, N]
    body_rev[b,c,j] = pos_body[c,j] + m[b,j]*(neg_body-pos_body)[c,j]
    body_min[b,c]   = 1 - max_j body_rev[b,c,j]
    lb[b,n] = max_c body_min[b,c]*pos_head[c,n]
    ub[b,n] = 1 - max_c body_min[b,c]*neg_head[c,n]
    updated = clamp(m, min(lb,ub), max(lb,ub))
    out = preds with columns `atoms` replaced by updated

Min-form rewrite: body_min = min over body literals of (m_j if pos literal,
1-m_j if neg literal), padded with 1.0.  Each packed literal carries
RELATIVE error <= 2^-9 in bf16, and min/max preserve relative error, so the
result is within ~4e-3 relative of the fp32 reference (empirically 3.9e-3,
gate is 2e-2).

Device pipeline per core (all 128 batch rows on partitions):
  1. DMA packed literal matrix G [128, COLS] bf16 (slots sorted by body
     width, widths bucketed to multiples of 4; pad value 1.0; dummy slots
     0.0) in chunks overlapped with stage-1.
  2. Stage-1 (DVE): per width-bucket strided tensor_reduce(min) -> bmin
     [128, S_pad] bf16 (one value per constraint slot).
  3. PE transpose of bmin -> bminT, then PE matmul with a one-hot matrix
     P[slot, cell] (cell = (sign, local atom, member)) -> PSUM [128, 1024]
     fp32.  This scatters each constraint's body_min into its head bin with
     arbitrary permutation; empty cells stay 0 (neutral for max since
     body_min >= 0).
  4. DVE log-tree max sweeps over member dim (8->4->2->1) -> lb/ubm
     aligned to local atom order, then the clamp chain in fp32 with exact
     m values.
  5. DMA out updated [128, 64] fp32; host scatters into preds.

Sharding: atoms dealt greedily (heaviest literal load first) to 8 cores;
all cores share one SPMD program (bucket counts padded to cross-core max).
"""

import sys
from contextlib import ExitStack

import numpy as np

if "/opt/trn_rl_repo" not in sys.path:
    sys.path.insert(0, "/opt/trn_rl_repo")

import concourse.bacc as bacc
import concourse.tile as tile
from concourse import mybir
from concourse.bass_utils import run_bass_kernel_spmd
from concourse.masks import make_identity

B = 128
C = 1024
N = 512
NCORES = 8
NA = N // NCORES          # atoms per core = 64
W_BIN = 8                 # max head-bin size (max over data is 8)
CELLS = 2 * NA * W_BIN    # 1024
N_CHUNKS = 4

_TRACE = False
_LAST_RESULTS = None
_PROGRAM_CACHE: dict = {}

_BF_NP = mybir.dt.np(mybir.dt.bfloat16)


def _build_program(key):
    """key = (chunks, s_pad, cols) where chunks is a tuple of
    (c0, c1, reduces) and reduces is a tuple of (goff, nslots, w, boff)."""
    if key in _PROGRAM_CACHE:
        return _PROGRAM_CACHE[key]
    chunks, s_pad, cols = key
    s1 = s_pad - 128
    assert 0 < s1 <= 128

    dt = mybir.dt
    nc = bacc.Bacc(
        "TRN2", target_bir_lowering=False, debug=False, enable_partition_id=False
    )
    c_ds = [
        nc.dram_tensor(f"c{i}", [B, c1 - c0], dt.bfloat16, kind="ExternalInput")
        for i, (c0, c1, _) in enumerate(chunks)
    ]
    p0_d = nc.dram_tensor("p0", [128, CELLS], dt.bfloat16, kind="ExternalInput")
    p1_d = nc.dram_tensor("p1", [s1, CELLS], dt.bfloat16, kind="ExternalInput")
    mloc_d = nc.dram_tensor("mloc", [B, NA], dt.float32, kind="ExternalInput")
    out_d = nc.dram_tensor("upd", [B, NA], dt.float32, kind="ExternalOutput")

    with ExitStack() as ctx:
        tc = ctx.enter_context(tile.TileContext(nc))
        pool = ctx.enter_context(tc.tile_pool(name="main", bufs=1))
        psum = ctx.enter_context(tc.tile_pool(name="ps", bufs=1, space="PSUM"))

        ident = pool.tile([128, 128], dt.bfloat16, tag="ident")
        make_identity(nc, ident[:])

        mloc_sb = pool.tile([B, NA], dt.float32, tag="mloc")
        nc.sync.dma_start(mloc_sb[:], mloc_d.ap())
        p0_sb = pool.tile([128, CELLS], dt.bfloat16, tag="p0")
        nc.sync.dma_start(p0_sb[:], p0_d.ap())
        p1_sb = pool.tile([s1, CELLS], dt.bfloat16, tag="p1")
        nc.sync.dma_start(p1_sb[:], p1_d.ap())

        g_sb = pool.tile([B, cols], dt.bfloat16, tag="g")
        bmin_sb = pool.tile([B, s_pad], dt.bfloat16, tag="bmin")
        dma_engines = [nc.scalar, nc.gpsimd]
        for i, (c0, c1, reduces) in enumerate(chunks):
            dma_engines[i % 2].dma_start(g_sb[:, c0:c1], c_ds[i].ap())
            for goff, nslots, w, boff in reduces:
                g3 = g_sb[:, goff : goff + nslots * w].rearrange(
                    "p (c k) -> p c k", k=w
                )
                nc.vector.tensor_reduce(
                    bmin_sb[:, boff : boff + nslots], g3,
                    axis=mybir.AxisListType.X, op=mybir.AluOpType.min,
                )

        # Scatter body_min -> head-bin cells via PE (transpose + one-hot matmul)
        pt0 = psum.tile([128, 128], dt.bfloat16, tag="pt0")
        nc.tensor.transpose(pt0[:], bmin_sb[:, 0:128], ident[:])
        bt0 = pool.tile([128, 128], dt.bfloat16, tag="bt0")
        nc.scalar.copy(bt0[:], pt0[:])
        pt1 = psum.tile([s1, 128], dt.bfloat16, tag="pt1")
        nc.tensor.transpose(pt1[:], bmin_sb[:, 128:s_pad], ident[:])
        bt1 = pool.tile([s1, 128], dt.bfloat16, tag="bt1")
        nc.scalar.copy(bt1[:], pt1[:])

        po = psum.tile([B, CELLS], dt.float32, tag="po")
        for nt in range(0, CELLS, 512):
            nc.tensor.matmul(
                po[:, nt : nt + 512], bt0[:], p0_sb[:, nt : nt + 512],
                start=True, stop=False,
            )
            nc.tensor.matmul(
                po[:, nt : nt + 512], bt1[:], p1_sb[:, nt : nt + 512],
                start=False, stop=True,
            )

        # Max sweeps over member dim: [B, 2*NA, 8] -> [B, 2*NA].  Only one
        # tensor_tensor input may come from PSUM, so stage the hi half in SBUF.
        po3 = po[:].rearrange("p (c k) -> p c k", k=W_BIN)
        s8h = pool.tile([B, 2 * NA, 4], dt.float32, tag="s8h")
        nc.vector.tensor_copy(s8h[:], po3[:, :, 4:8])
        s4 = pool.tile([B, 2 * NA, 4], dt.float32, tag="s4")
        nc.vector.tensor_tensor(
            s4[:], po3[:, :, 0:4], s8h[:], op=mybir.AluOpType.max
        )
        s2 = pool.tile([B, 2 * NA, 2], dt.float32, tag="s2")
        nc.vector.tensor_tensor(
            s2[:], s4[:, :, 0:2], s4[:, :, 2:4], op=mybir.AluOpType.max
        )
        lbub = pool.tile([B, 2 * NA], dt.float32, tag="lbub")
        lbub3 = lbub[:].rearrange("p (c k) -> p c k", k=1)
        nc.vector.tensor_tensor(
            lbub3, s2[:, :, 0:1], s2[:, :, 1:2], op=mybir.AluOpType.max
        )

        # updated = max(min(lb, ub), min(max(lb, ub), m)),  ub = 1 - ubm
        lb = lbub[:, 0:NA]
        ubm = lbub[:, NA : 2 * NA]
        ub_sb = pool.tile([B, NA], dt.float32, tag="ub")
        nc.vector.tensor_scalar(
            ub_sb[:], ubm, -1.0, 1.0,
            op0=mybir.AluOpType.mult, op1=mybir.AluOpType.add,
        )
        lo_sb = pool.tile([B, NA], dt.float32, tag="lo")
        nc.vector.tensor_tensor(lo_sb[:], lb, ub_sb[:], op=mybir.AluOpType.min)
        hi_sb = pool.tile([B, NA], dt.float32, tag="hi")
        nc.vector.tensor_tensor(hi_sb[:], lb, ub_sb[:], op=mybir.AluOpType.max)
        upd_sb = pool.tile([B, NA], dt.float32, tag="upd")
        nc.vector.tensor_tensor(
            upd_sb[:], hi_sb[:], mloc_sb[:], op=mybir.AluOpType.min
        )
        nc.vector.tensor_tensor(
            upd_sb[:], lo_sb[:], upd_sb[:], op=mybir.AluOpType.max
        )
        nc.sync.dma_start(out_d.ap(), upd_sb[:])

    nc.compile()
    _PROGRAM_CACHE[key] = nc
    return nc


def kernel(preds, pos_head, neg_head, pos_body, neg_body, atoms):
    global _LAST_RESULTS
    preds = np.ascontiguousarray(np.asarray(preds, dtype=np.float32))
    pos_head = np.asarray(pos_head)
    neg_head = np.asarray(neg_head)
    pos_body = np.asarray(pos_body)
    neg_body = np.asarray(neg_body)
    atoms_np = np.asarray(atoms).astype(np.int64)

    m = np.ascontiguousarray(preds[:, atoms_np].astype(np.float32))  # [B, N]
    rev = (np.float32(1.0) - m).astype(np.float32)
    # literal column space: [0,N) = m, [N,2N) = 1-m, 2N = 0.0 (dummy fill),
    # 2N+1 = 1.0 (real-slot width pad)
    mext = np.concatenate(
        [m, rev, np.zeros((B, 1), np.float32), np.ones((B, 1), np.float32)],
        axis=1,
    ).astype(_BF_NP)
    DUMMY_COL, PAD_COL = 2 * N, 2 * N + 1

    pb = pos_body != 0
    nb_ = neg_body != 0
    # per-constraint literal cols (pos as m, neg as 1-m) and bucketed width
    lit_cols = []
    wb = np.zeros(C, np.int64)
    for c in range(C):
        jp = np.nonzero(pb[c])[0]
        jn = np.nonzero(nb_[c])[0]
        lit_cols.append(np.concatenate([jp, N + jn]))
        wb[c] = max(-(-(jp.size + jn.size) // 4) * 4, 4)

    ph_atom = pos_head.argmax(1)
    ph_has = pos_head.max(1) > 0
    nh_atom = neg_head.argmax(1)
    nh_has = neg_head.max(1) > 0
    pos_bins = [[] for _ in range(N)]
    neg_bins = [[] for _ in range(N)]
    for c in np.nonzero(ph_has)[0]:
        pos_bins[ph_atom[c]].append(c)
    for c in np.nonzero(nh_has)[0]:
        neg_bins[nh_atom[c]].append(c)

    # Greedy deal: heaviest atoms (by bucketed literal load) first, to the
    # least-loaded core with room.
    weight = np.array(
        [sum(wb[c] for c in pos_bins[a] + neg_bins[a]) for a in range(N)]
    )
    order = np.argsort(-weight, kind="stable")
    core_load = np.zeros(NCORES, np.int64)
    core_atoms = [[] for _ in range(NCORES)]
    for a in order:
        cands = [k for k in range(NCORES) if len(core_atoms[k]) < NA]
        k = min(cands, key=lambda k: core_load[k])
        core_atoms[k].append(int(a))
        core_load[k] += weight[a]

    # Per-core slots grouped by width bucket; cross-core max count per bucket.
    from collections import defaultdict

    core_buckets = []  # per core: {wb: [(cols_array, cell), ...]}
    for k in range(NCORES):
        bk = defaultdict(list)
        for a_local, a in enumerate(core_atoms[k]):
            for member, c in enumerate(pos_bins[a]):
                bk[int(wb[c])].append((lit_cols[c], a_local * W_BIN + member))
            for member, c in enumerate(neg_bins[a]):
                bk[int(wb[c])].append(
                    (lit_cols[c], NA * W_BIN + a_local * W_BIN + member)
                )
        core_buckets.append(bk)
    all_w = sorted(
        {w for bk in core_buckets for w in bk}, reverse=True
    )
    bucket_cnt = {
        w: max(len(bk.get(w, ())) for bk in core_buckets) for w in all_w
    }
    s_pad = sum(bucket_cnt.values())
    s_pad = max(s_pad, 132)  # ensure a nonempty second K-tile
    extra = s_pad - sum(bucket_cnt.values())
    bucket_cnt[all_w[-1]] += extra
    cols = sum(bucket_cnt[w] * w for w in all_w)

    # Chunk boundaries at slot boundaries, ~equal bytes; reduces split per
    # (bucket x chunk) so each reduce starts as soon as its chunk lands.
    slot_edges = []  # (gend, w, bucket_slot_idx_end)
    goff = 0
    for w in all_w:
        for i in range(bucket_cnt[w]):
            goff += w
            slot_edges.append(goff)
    cuts = [0]
    for t in range(1, N_CHUNKS):
        target = cols * t // N_CHUNKS
        cuts.append(min(slot_edges, key=lambda e: abs(e - target)))
    cuts.append(cols)
    cuts = sorted(set(cuts))

    chunks = []
    goff = 0
    boff = 0
    ci = 0
    cur_reduces = []
    cur_c0 = 0
    for w in all_w:
        nsl = bucket_cnt[w]
        s = 0
        while s < nsl:
            cur_end = cuts[ci + 1]
            take = min((cur_end - goff) // w, nsl - s)
            if take > 0:
                cur_reduces.append((goff, take, w, boff))
                goff += take * w
                boff += take
                s += take
            if goff >= cur_end and ci + 2 <= len(cuts) - 1:
                chunks.append((cur_c0, goff, tuple(cur_reduces)))
                cur_c0 = goff
                cur_reduces = []
                ci += 1
    chunks.append((cur_c0, cols, tuple(cur_reduces)))
    chunks = tuple(c for c in chunks if c[1] > c[0])

    key = (chunks, s_pad, cols)
    nc = _build_program(key)

    s1 = s_pad - 128
    in_maps = []
    for k in range(NCORES):
        col_idx = np.full(cols, DUMMY_COL, np.int32)
        P = np.zeros((s_pad, CELLS), _BF_NP)
        goff = 0
        si = 0
        for w in all_w:
            slots = core_buckets[k].get(w, [])
            for j in range(bucket_cnt[w]):
                if j < len(slots):
                    lc, cell = slots[j]
                    col_idx[goff : goff + lc.size] = lc
                    col_idx[goff + lc.size : goff + w] = PAD_COL
                    P[si, cell] = 1.0
                goff += w
                si += 1
        g = np.ascontiguousarray(mext[:, col_idx])
        im = {
            "p0": np.ascontiguousarray(P[0:128]),
            "p1": np.ascontiguousarray(P[128:s_pad]),
            "mloc": np.ascontiguousarray(m[:, core_atoms[k]]),
        }
        for i, (c0, c1, _) in enumerate(chunks):
            im[f"c{i}"] = np.ascontiguousarray(g[:, c0:c1])
        in_maps.append(im)

    res = run_bass_kernel_spmd(
        nc, in_maps, core_ids=list(range(NCORES)), trace=_TRACE
    )
    _LAST_RESULTS = res

    out = preds.copy()
    for k in range(NCORES):
        out[:, atoms_np[core_atoms[k]]] = res.results[k]["upd"]
    return out


# revision 4
# speedup vs baseline: 1.4880x; 1.1178x over previous
"""Trainium2 Bass kernel for nn_ConstraintsModule (v3).

Reference math:
    m = preds[:, atoms]                                   # [B, N]
    body_rev[b,c,j] = pos_body[c,j] + m[b,j]*(neg_body-pos_body)[c,j]
    body_min[b,c]   = 1 - max_j body_rev[b,c,j]
    lb[b,n] = max_c body_min[b,c]*pos_head[c,n]
    ub[b,n] = 1 - max_c body_min[b,c]*neg_head[c,n]
    updated = clamp(m, min(lb,ub), max(lb,ub))
    out = preds with columns `atoms` replaced by updated

Min-form rewrite: body_min = min over body literals of (m_j if pos literal,
1-m_j if neg literal), padded with 1.0.  Packed literals carry RELATIVE
error <= 2^-9 in bf16 and min/max preserve relative error, so the result is
within ~4e-3 relative of the fp32 reference (gate 2e-2).

Device pipeline per core (batch rows on partitions):
  1. DMA packed literal matrix G [128, COLS] bf16 (slots sorted by body
     width, widths bucketed by a cost-optimal DP; pad 1.0, dummy slots 0.0)
     in chunks, smallest first, overlapped with stage 1.
  2. Stage 1 (DVE): per width-bucket strided tensor_reduce(min) -> bmin
     [128, S_pad] bf16.
  3. PE transpose of bmin (two K-tiles) + matmul with one-hot P[slot,cell]
     -> PSUM [128, 528] fp32, cell = member*132 + sign*66 + local atom.
     Head bins capped at 4 members; the four oversized bins in this data
     spill members 5..8 into donor columns 64/65, folded back by one max.
  4. DVE max sweeps (member-major, contiguous) -> lb/ubm, then the fp32
     clamp chain with exact m values; DMA out updated [128, 66].
ACT engine pre-warms its table and does the PSUM->SBUF staging copies; PE
is pre-warmed with a dummy transpose while DMAs are in flight.

Sharding: 64 atoms per core, dealt greedily by literal load; all cores run
one SPMD program (bucket counts padded to the cross-core max).
"""

import sys
from contextlib import ExitStack

import numpy as np

if "/opt/trn_rl_repo" not in sys.path:
    sys.path.insert(0, "/opt/trn_rl_repo")

import concourse.bacc as bacc
import concourse.tile as tile
from concourse import mybir
from concourse.bass_utils import run_bass_kernel_spmd
from concourse.masks import make_identity

B = 128
C = 1024
N = 512
NCORES = 8
NA = N // NCORES + 2      # 64 real atoms + 2 donor columns = 66
W_BIN = 4
CELLS = W_BIN * 2 * NA    # 528, member-major: cell = member*132 + sign*66 + a
NBINS = 2 * NA            # 132

_TRACE = False
_LAST_RESULTS = None
_PROGRAM_CACHE: dict = {}

_BF_NP = mybir.dt.np(mybir.dt.bfloat16)


def _build_program(key):
    """key = (chunks, s_pad, cols); chunks = tuple of (c0, c1, reduces),
    reduces = tuple of (goff, nslots, w, boff)."""
    if key in _PROGRAM_CACHE:
        return _PROGRAM_CACHE[key]
    chunks, s_pad, cols = key
    s1 = s_pad - 128
    assert 0 < s1 <= 128

    dt = mybir.dt
    nc = bacc.Bacc(
        "TRN2", target_bir_lowering=False, debug=False, enable_partition_id=False
    )
    c_ds = [
        nc.dram_tensor(f"c{i}", [B, c1 - c0], dt.bfloat16, kind="ExternalInput")
        for i, (c0, c1, _) in enumerate(chunks)
    ]
    p0_d = nc.dram_tensor("p0", [128, CELLS], dt.bfloat16, kind="ExternalInput")
    p1_d = nc.dram_tensor("p1", [s1, CELLS], dt.bfloat16, kind="ExternalInput")
    mloc_d = nc.dram_tensor("mloc", [B, NA], dt.float32, kind="ExternalInput")
    out_d = nc.dram_tensor("upd", [B, NA], dt.float32, kind="ExternalOutput")

    with ExitStack() as ctx:
        tc = ctx.enter_context(tile.TileContext(nc))
        pool = ctx.enter_context(tc.tile_pool(name="main", bufs=1))
        psum = ctx.enter_context(tc.tile_pool(name="ps", bufs=1, space="PSUM"))

        g_sb = pool.tile([B, cols], dt.bfloat16, tag="g")
        p0_sb = pool.tile([128, CELLS], dt.bfloat16, tag="p0")
        p1_sb = pool.tile([s1, CELLS], dt.bfloat16, tag="p1")
        mloc_sb = pool.tile([B, NA], dt.float32, tag="mloc")
        bmin_sb = pool.tile([B, s_pad], dt.bfloat16, tag="bmin")
        ident = pool.tile([128, 128], dt.bfloat16, tag="ident")
        warm = pool.tile([B, 8], dt.float32, tag="warm")

        # First instructions per engine: get the G DMAs issued immediately.
        dma_engines = [nc.gpsimd, nc.scalar]
        reduce_parts = []
        for i, (c0, c1, reduces) in enumerate(chunks):
            dma_engines[i % 2].dma_start(g_sb[:, c0:c1], c_ds[i].ap())
            reduce_parts.append(reduces)
        nc.sync.dma_start(p0_sb[:], p0_d.ap())
        nc.sync.dma_start(p1_sb[:], p1_d.ap())
        nc.sync.dma_start(mloc_sb[:], mloc_d.ap())

        # Warm-ups while DMAs fly: ACT table load, PE p-state, identity.
        make_identity(nc, ident[:])
        nc.scalar.copy(warm[:], warm[:])
        wpt = psum.tile([8, 128], dt.bfloat16, tag="wpt")
        nc.tensor.transpose(wpt[:], ident[:, 0:8], ident[:])

        for i, (c0, c1, _) in enumerate(chunks):
            for goff, nslots, w, boff in reduce_parts[i]:
                g3 = g_sb[:, goff : goff + nslots * w].rearrange(
                    "p (c k) -> p c k", k=w
                )
                nc.vector.tensor_reduce(
                    bmin_sb[:, boff : boff + nslots], g3,
                    axis=mybir.AxisListType.X, op=mybir.AluOpType.min,
                )

        # PE scatter: bminT (two K-tiles) @ one-hot P -> cells in PSUM.
        pt0 = psum.tile([128, 128], dt.bfloat16, tag="pt0")
        nc.tensor.transpose(pt0[:], bmin_sb[:, 0:128], ident[:])
        bt0 = pool.tile([128, 128], dt.bfloat16, tag="bt0")
        nc.scalar.copy(bt0[:], pt0[:])
        pt1 = psum.tile([s1, 128], dt.bfloat16, tag="pt1")
        nc.tensor.transpose(pt1[:], bmin_sb[:, 128:s_pad], ident[:])
        bt1 = pool.tile([s1, 128], dt.bfloat16, tag="bt1")
        nc.scalar.copy(bt1[:], pt1[:])

        po = psum.tile([B, CELLS], dt.float32, tag="po")
        for nt0 in range(0, CELLS, 512):
            nt1 = min(nt0 + 512, CELLS)
            nc.tensor.matmul(
                po[:, nt0:nt1], bt0[:], p0_sb[:, nt0:nt1], start=True, stop=False
            )
            nc.tensor.matmul(
                po[:, nt0:nt1], bt1[:], p1_sb[:, nt0:nt1], start=False, stop=True
            )

        # Member-major max sweeps: [B, 4*132] -> [B, 132], all contiguous.
        # ACT stages members 2..3 into SBUF (one PSUM input max per op).
        HALF = 2 * NBINS  # 264
        s_hi = pool.tile([B, HALF], dt.float32, tag="s_hi")
        nc.scalar.copy(s_hi[:], po[:, HALF : 2 * HALF])
        s4 = pool.tile([B, HALF], dt.float32, tag="s4")
        nc.vector.tensor_tensor(
            s4[:], po[:, 0:HALF], s_hi[:], op=mybir.AluOpType.max
        )
        lbub = pool.tile([B, NBINS], dt.float32, tag="lbub")
        nc.vector.tensor_tensor(
            lbub[:], s4[:, 0:NBINS], s4[:, NBINS:HALF], op=mybir.AluOpType.max
        )
        # Fold donor columns (oversized-bin spill) back into atoms 0/1.
        nc.vector.tensor_tensor(
            lbub[:, 0:2], lbub[:, 0:2], lbub[:, NA - 2 : NA],
            op=mybir.AluOpType.max,
        )

        # updated = max(min(lb, ub), min(max(lb, ub), m)),  ub = 1 - ubm
        lb = lbub[:, 0:NA]
        ubm = lbub[:, NA:NBINS]
        ub_sb = pool.tile([B, NA], dt.float32, tag="ub")
        nc.vector.tensor_scalar(
            ub_sb[:], ubm, -1.0, 1.0,
            op0=mybir.AluOpType.mult, op1=mybir.AluOpType.add,
        )
        lo_sb = pool.tile([B, NA], dt.float32, tag="lo")
        nc.vector.tensor_tensor(lo_sb[:], lb, ub_sb[:], op=mybir.AluOpType.min)
        hi_sb = pool.tile([B, NA], dt.float32, tag="hi")
        nc.vector.tensor_tensor(hi_sb[:], lb, ub_sb[:], op=mybir.AluOpType.max)
        upd_sb = pool.tile([B, NA], dt.float32, tag="upd")
        nc.vector.tensor_tensor(
            upd_sb[:], hi_sb[:], mloc_sb[:], op=mybir.AluOpType.min
        )
        nc.vector.tensor_tensor(
            upd_sb[:], lo_sb[:], upd_sb[:], op=mybir.AluOpType.max
        )
        nc.sync.dma_start(out_d.ap(), upd_sb[:])

    nc.compile()
    _PROGRAM_CACHE[key] = nc
    return nc


def _plan_buckets(kcounts):
    """DP over width cut points minimizing DVE cost: padded columns * 1.04ns
    (per core ~ /8) + ~155ns per reduce instruction."""
    ws = sorted(kcounts)  # distinct exact widths (multiples of 4 upstream)
    nw = len(ws)
    suffix_cnt = [0] * (nw + 1)
    for i in range(nw - 1, -1, -1):
        suffix_cnt[i] = suffix_cnt[i + 1] + kcounts[ws[i]]
    best = {}  # i -> (cost, cuts)

    def solve(i):
        if i >= nw:
            return (0.0, ())
        if i in best:
            return best[i]
        r = None
        for j in range(i, nw):  # bucket covers widths ws[i..j] padded to ws[j]
            cnt = suffix_cnt[i] - suffix_cnt[j + 1]
            cost = cnt * ws[j] * 1.04 / NCORES + 155.0
            sub = solve(j + 1)
            tot = cost + sub[0]
            if r is None or tot < r[0]:
                r = (tot, (ws[j],) + sub[1])
        best[i] = r
        return r

    return solve(0)[1]


def kernel(preds, pos_head, neg_head, pos_body, neg_body, atoms):
    global _LAST_RESULTS
    preds = np.ascontiguousarray(np.asarray(preds, dtype=np.float32))
    pos_head = np.asarray(pos_head)
    neg_head = np.asarray(neg_head)
    pos_body = np.asarray(pos_body)
    neg_body = np.asarray(neg_body)
    atoms_np = np.asarray(atoms).astype(np.int64)

    m = np.ascontiguousarray(preds[:, atoms_np].astype(np.float32))  # [B, N]
    rev = (np.float32(1.0) - m).astype(np.float32)
    # literal cols: [0,N) = m, [N,2N) = 1-m, 2N = 0.0 (dummy), 2N+1 = 1.0 (pad)
    mext = np.concatenate(
        [m, rev, np.zeros((B, 1), np.float32), np.ones((B, 1), np.float32)],
        axis=1,
    ).astype(_BF_NP)
    DUMMY_COL, PAD_COL = 2 * N, 2 * N + 1

    pb = pos_body != 0
    nb_ = neg_body != 0
    lit_cols = []
    kw = np.zeros(C, np.int64)  # exact width rounded to 4
    for c in range(C):
        jp = np.nonzero(pb[c])[0]
        jn = np.nonzero(nb_[c])[0]
        lit_cols.append(np.concatenate([jp, N + jn]))
        kw[c] = max(-(-(jp.size + jn.size) // 4) * 4, 4)

    from collections import Counter, defaultdict

    cuts = _plan_buckets(Counter(int(w) for w in kw))
    # wb[c] = bucketed width
    wb = np.zeros(C, np.int64)
    for c in range(C):
        wb[c] = next(w for w in sorted(cuts) if w >= kw[c])

    ph_atom = pos_head.argmax(1)
    ph_has = pos_head.max(1) > 0
    nh_atom = neg_head.argmax(1)
    nh_has = neg_head.max(1) > 0
    pos_bins = [[] for _ in range(N)]
    neg_bins = [[] for _ in range(N)]
    for c in np.nonzero(ph_has)[0]:
        pos_bins[ph_atom[c]].append(c)
    for c in np.nonzero(nh_has)[0]:
        neg_bins[nh_atom[c]].append(c)
    assert max(len(b) for b in neg_bins) <= W_BIN
    big_atoms = [a for a in range(N) if len(pos_bins[a]) > W_BIN]
    assert all(len(pos_bins[a]) <= 2 * W_BIN for a in big_atoms)

    # Greedy deal by bucketed literal load; big atoms first so they spread
    # across cores (<= 2 each) and land at local indices 0/1.
    weight = np.array(
        [sum(wb[c] for c in pos_bins[a] + neg_bins[a]) for a in range(N)]
    )
    order = sorted(range(N), key=lambda a: (a not in big_atoms, -weight[a]))
    core_load = np.zeros(NCORES, np.int64)
    core_atoms = [[] for _ in range(NCORES)]
    nbig = np.zeros(NCORES, np.int64)
    for a in order:
        big = a in big_atoms
        cands = [
            k for k in range(NCORES)
            if len(core_atoms[k]) < 64 and (not big or nbig[k] < 2)
        ]
        k = min(cands, key=lambda k: core_load[k])
        core_atoms[k].append(int(a))
        core_load[k] += weight[a]
        if big:
            nbig[k] += 1

    # Per-core slots by bucket.  cell = member*NBINS + sign*NA + a_local;
    # pos members 4..7 of big atoms go to donor column NA-2 + bigidx.
    core_buckets = []
    for k in range(NCORES):
        bk = defaultdict(list)
        bigidx = 0
        for a_local, a in enumerate(core_atoms[k]):
            for mem, c in enumerate(pos_bins[a]):
                if mem < W_BIN:
                    cell = mem * NBINS + a_local
                else:
                    assert a_local < 2
                    cell = (mem - W_BIN) * NBINS + (NA - 2 + a_local)
                bk[int(wb[c])].append((lit_cols[c], cell))
            for mem, c in enumerate(neg_bins[a]):
                bk[int(wb[c])].append((lit_cols[c], mem * NBINS + NA + a_local))
        core_buckets.append(bk)
    all_w = sorted({w for bk in core_buckets for w in bk}, reverse=True)
    bucket_cnt = {
        w: -(-max(len(bk.get(w, ())) for bk in core_buckets) // 2) * 2
        for w in all_w
    }
    s_pad = sum(bucket_cnt.values())
    if s_pad < 132:
        bucket_cnt[all_w[-1]] += 132 - s_pad
        s_pad = 132
    cols = sum(bucket_cnt[w] * w for w in all_w)

    # Chunks at slot boundaries: small first for an early DVE start.
    slot_edges = []
    goff = 0
    for w in all_w:
        for _ in range(bucket_cnt[w]):
            goff += w
            slot_edges.append(goff)
    fracs = (0.08, 0.36, 0.68)
    cutpts = sorted(
        {min(slot_edges, key=lambda e: abs(e - int(cols * f))) for f in fracs}
    )
    cutpts = [0] + [cp for cp in cutpts if 0 < cp < cols] + [cols]

    chunks = []
    goff = 0
    boff = 0
    ci = 0
    cur_reduces = []
    cur_c0 = 0
    for w in all_w:
        nsl = bucket_cnt[w]
        s = 0
        while s < nsl:
            take = min((cutpts[ci + 1] - goff) // w, nsl - s)
            if take > 0:
                cur_reduces.append((goff, take, w, boff))
                goff += take * w
                boff += take
                s += take
            if goff >= cutpts[ci + 1] and ci + 2 <= len(cutpts) - 1:
                chunks.append((cur_c0, goff, tuple(cur_reduces)))
                cur_c0 = goff
                cur_reduces = []
                ci += 1
    chunks.append((cur_c0, cols, tuple(cur_reduces)))
    chunks = tuple(c for c in chunks if c[1] > c[0])

    key = (chunks, s_pad, cols)
    nc = _build_program(key)

    in_maps = []
    for k in range(NCORES):
        col_idx = np.full(cols, DUMMY_COL, np.int32)
        P = np.zeros((s_pad, CELLS), _BF_NP)
        goff = 0
        si = 0
        for w in all_w:
            slots = core_buckets[k].get(w, [])
            for j in range(bucket_cnt[w]):
                if j < len(slots):
                    lc, cell = slots[j]
                    col_idx[goff : goff + lc.size] = lc
                    col_idx[goff + lc.size : goff + w] = PAD_COL
                    P[si, cell] = 1.0
                goff += w
                si += 1
        g = np.ascontiguousarray(mext[:, col_idx])
        ml = np.zeros((B, NA), np.float32)
        ml[:, 0:64] = m[:, core_atoms[k]]
        im = {
            "p0": np.ascontiguousarray(P[0:128]),
            "p1": np.ascontiguousarray(P[128:s_pad]),
            "mloc": ml,
        }
        for i, (c0, c1, _) in enumerate(chunks):
            im[f"c{i}"] = np.ascontiguousarray(g[:, c0:c1])
        in_maps.append(im)

    res = run_bass_kernel_spmd(
        nc, in_maps, core_ids=list(range(NCORES)), trace=_TRACE
    )
    _LAST_RESULTS = res

    out = preds.copy()
    for k in range(NCORES):
        out[:, atoms_np[core_atoms[k]]] = res.results[k]["upd"][:, 0:64]
    return out
